# revision 5
# baseline (speedup 1.0000x reference)
"""DiT block kernel for 8 TRN2 NeuronCores.

Sharding: token-parallel. B*N = 4096 tokens split 512/core; cores 0-3 own
batch 0, cores 4-7 batch 1. LayerNorms, projections and the MLP are purely
token-local; attention needs each batch's full K/V, obtained with one
AllGather per tensor inside each 4-core group.

On-chip layout: activations are kept feature-major ([C, tokens], features on
partitions) so every matmul consumes the previous one's output directly
(out = lhsT.T @ rhs with weights stationary). V is produced token-major so it
can serve as the stationary operand of the P^T @ V matmul. Attention scores
are computed transposed (S^T = K Q^T, [keys, queries]) which makes the
softmax denominator a partition-axis sum that we fold into the O matmul as a
65th weight column of ones. exp() needs no max-subtraction: |scores| < ~3 by
construction of the problem's scales.

All matmuls run in bf16 with fp32 PSUM accumulation; LayerNorm statistics and
residuals stay fp32.

SBUF tags reuse one slot across disjoint-lifetime tensors:
  A: wqkv(48K) -> v_full(33K) -> h1g(32K)
  C: k_full(32K) -> y(16K)
  X: x(16K) -> h2act(8K)
  E: h1act(8K) -> attn(8K)
  F: v_sb(8.3K) -> q_all(8K)
"""

import os
import sys
import numpy as np

for _p in ("/opt/trn_rl_repo", "/root/.axon_site/_ro/trn_rl_repo"):
    if os.path.isdir(_p) and _p not in sys.path:
        sys.path.insert(0, _p)

import ml_dtypes  # noqa: E402

BF = ml_dtypes.bfloat16

NCORES = 8
GROUP = 4          # cores per batch (all-gather group)
B, N, C = 2, 2048, 1024
H, D = 16, 64
F = 4096
NT = (B * N) // NCORES   # 512 tokens per core
KT = C // 128            # 8 k-tiles over the model dim
TT = NT // 128           # 4 token tiles per core
FT = F // 128            # 32 tiles over d_ff
HP = H // 2              # 8 head-pair tiles (2 heads of 64 = 128 partitions)
NKT = (GROUP * NT) // 128  # 16 key tiles per batch
EPS = 1e-6

_CACHE = {}
LAST_RESULT = None


def _build_program():
    import concourse.tile as tile
    import concourse.mybir as mybir
    from concourse import bacc
    from contextlib import ExitStack

    f32 = mybir.dt.float32
    bf16 = mybir.dt.bfloat16
    AF = mybir.ActivationFunctionType
    OP = mybir.AluOpType

    nc = bacc.Bacc("TRN2", target_bir_lowering=False, debug=False,
                   num_devices=NCORES)

    def din(name, shape, dt):
        return nc.dram_tensor(name, shape, dt, kind="ExternalInput")

    xT = din("xT", [KT, 128, NT], f32)
    cosr = din("cosr", [128, NT], bf16)
    sinr = din("sinr", [128, NT], bf16)
    pswap = din("pswap", [128, 128], bf16)
    wqkv = din("wqkv", [KT, 128, 3 * C], bf16)
    wo = din("wo", [KT, KT, 128, 128], bf16)    # [mt, kt, p, cols]
    w1 = din("w1", [FT, KT, 128, 128], bf16)    # [mt, kt, p, cols]
    w2 = din("w2", [KT, FT, 128, 128], bf16)    # [mt, kt, p, cols]
    bqk = din("bqk", [2, KT, 128, 1], f32)      # [0]=bq*0.125, [1]=bk
    bvr = din("bvr", [1, C], bf16)
    bo_c = din("bo", [KT, 128, 1], f32)
    b1_c = din("b1", [FT, 128, 1], f32)
    b2_c = din("b2", [KT, 128, 1], f32)
    g1_c = din("g1", [KT, 128, 1], f32)
    h1_c = din("h1", [KT, 128, 1], f32)         # ln1 shift
    g2_c = din("g2", [KT, 128, 1], f32)
    h2_c = din("h2", [KT, 128, 1], f32)         # ln2 shift

    outT = nc.dram_tensor("outT", [KT, 128, NT], f32, kind="ExternalOutput")

    groups = [list(range(g * GROUP, (g + 1) * GROUP)) for g in range(NCORES // GROUP)]

    with tile.TileContext(nc) as tc, ExitStack() as ctx:
        sb = ctx.enter_context(tc.tile_pool(name="sb", bufs=1))
        tmp = ctx.enter_context(tc.tile_pool(name="tmp", bufs=2))
        wst = ctx.enter_context(tc.tile_pool(name="wst", bufs=2))
        pbuf = ctx.enter_context(tc.tile_pool(name="pbuf", bufs=4))
        ps_acc = ctx.enter_context(tc.tile_pool(name="ps_acc", bufs=2, space="PSUM"))
        ps_stat = ctx.enter_context(tc.tile_pool(name="ps_stat", bufs=1, space="PSUM"))
        ps_b = ctx.enter_context(tc.tile_pool(name="ps_b", bufs=1, space="PSUM"))
        ps_o = ctx.enter_context(tc.tile_pool(name="ps_o", bufs=2, space="PSUM"))
        dram = ctx.enter_context(tc.tile_pool(name="dram", bufs=1, space="DRAM"))

        # ---- constants in SBUF ----
        cos_sb = sb.tile([128, NT], bf16, tag="cos")
        sin_sb = sb.tile([128, NT], bf16, tag="sin")
        psw_sb = sb.tile([128, 128], bf16, tag="psw")
        nc.sync.dma_start(out=cos_sb[:], in_=cosr[:])
        nc.sync.dma_start(out=sin_sb[:], in_=sinr[:])
        nc.sync.dma_start(out=psw_sb[:], in_=pswap[:])
        ones_c = sb.tile([128, 1], bf16, tag="ones_c")
        nc.vector.memset(ones_c[:], 1.0)
        ones_r = sb.tile([1, 128], bf16, tag="ones_r")
        nc.vector.memset(ones_r[:], 1.0)
        ones_rf = sb.tile([1, 128], f32, tag="ones_rf")
        nc.vector.memset(ones_rf[:], 1.0)
        eps_sb = sb.tile([1, 1], f32, tag="eps")
        nc.vector.memset(eps_sb[:], EPS)

        def load_cols(name, src, n_out):
            t = sb.tile([128, n_out, 1], f32, tag=name)
            for i in range(n_out):
                nc.sync.dma_start(out=t[:, i, :], in_=src[i])
            return t

        bq_sb = load_cols("bq", bqk[0], KT)
        bk_sb = load_cols("bk", bqk[1], KT)
        bo_sb = load_cols("bo", bo_c, KT)
        b1_sb = load_cols("b1", b1_c, FT)
        b2_sb = load_cols("b2", b2_c, KT)
        g1_sb = load_cols("g1", g1_c, KT)
        h1_sb = load_cols("h1", h1_c, KT)
        g2_sb = load_cols("g2", g2_c, KT)
        h2_sb = load_cols("h2", h2_c, KT)
        bv_sb = sb.tile([1, C], bf16, tag="bv")
        nc.sync.dma_start(out=bv_sb[:], in_=bvr[:])

        # ---- weights: QKV resident (slot A) ----
        wqkv_sb = sb.tile([128, KT, 3 * C], bf16, tag="A")
        for kt in range(KT):
            nc.sync.dma_start(out=wqkv_sb[:, kt, :], in_=wqkv[kt])

        # ---- x in (slot X) ----
        x_sb = sb.tile([128, KT, NT], f32, tag="X")
        for kt in range(KT):
            nc.sync.dma_start(out=x_sb[:, kt, :], in_=xT[kt])

        # ---- layernorm (feature-major) ----
        def layernorm(src, g_col, b_col, out_tag):
            h_bf = sb.tile([128, KT, NT], bf16, tag=out_tag)
            sum_ps = ps_stat.tile([1, NT], f32, tag="s0")
            sq_ps = ps_stat.tile([1, NT], f32, tag="s1")
            for kt in range(KT):
                xb = tmp.tile([128, NT], bf16, tag="ln_xb")
                nc.scalar.copy(out=xb[:], in_=src[:, kt, :])
                sq = tmp.tile([128, NT], bf16, tag="ln_sq")
                nc.vector.tensor_mul(out=sq[:], in0=xb[:], in1=xb[:])
                nc.tensor.matmul(sum_ps[:], ones_c[:], xb[:],
                                 start=(kt == 0), stop=(kt == KT - 1))
                nc.tensor.matmul(sq_ps[:], ones_c[:], sq[:],
                                 start=(kt == 0), stop=(kt == KT - 1))
            mean = tmp.tile([1, NT], f32, tag="ln_mean")
            nc.vector.tensor_scalar_mul(out=mean[:], in0=sum_ps[:], scalar1=1.0 / C)
            ex2 = tmp.tile([1, NT], f32, tag="ln_ex2")
            nc.vector.tensor_scalar_mul(out=ex2[:], in0=sq_ps[:], scalar1=1.0 / C)
            var = tmp.tile([1, NT], f32, tag="ln_var")
            # var = ex2 - mean*mean  ==  ex2 - ((mean * -1) * mean)
            nc.vector.scalar_tensor_tensor(out=var[:], in0=mean[:], scalar=-1.0,
                                           in1=mean[:], op0=OP.mult, op1=OP.mult)
            nc.vector.tensor_add(out=var[:], in0=ex2[:], in1=var[:])
            std = tmp.tile([1, NT], f32, tag="ln_std")
            nc.scalar.activation(out=std[:], in_=var[:], func=AF.Sqrt,
                                 bias=eps_sb[:], scale=1.0)
            rstd = tmp.tile([1, NT], f32, tag="ln_rstd")
            nc.vector.reciprocal(out=rstd[:], in_=std[:])
            mean_b = ps_b.tile([128, NT], f32, tag="b0")
            rstd_b = ps_b.tile([128, NT], f32, tag="b1")
            nc.tensor.matmul(mean_b[:], ones_rf[:], mean[:], start=True, stop=True)
            nc.tensor.matmul(rstd_b[:], ones_rf[:], rstd[:], start=True, stop=True)
            for kt in range(KT):
                t0 = tmp.tile([128, NT], f32, tag="ln_t0")
                nc.vector.tensor_sub(out=t0[:], in0=src[:, kt, :], in1=mean_b[:])
                nc.vector.scalar_tensor_tensor(out=t0[:], in0=t0[:],
                                               scalar=g_col[:, kt, :], in1=rstd_b[:],
                                               op0=OP.mult, op1=OP.mult)
                nc.vector.tensor_scalar_add(out=h_bf[:, kt, :], in0=t0[:],
                                            scalar1=b_col[:, kt, :])
            return h_bf

        h_bf = layernorm(x_sb, g1_sb, h1_sb, "E")

        # ---- rope helper: out = raw*cos + (pswap@raw)*sin ----
        def rope(raw, out_ap):
            sw_ps = ps_acc.tile([128, NT], f32, tag="mm")
            nc.tensor.matmul(sw_ps[:], psw_sb[:], raw[:], start=True, stop=True)
            t1 = tmp.tile([128, NT], bf16, tag="rope_t1")
            nc.vector.tensor_mul(out=t1[:], in0=raw[:], in1=cos_sb[:])
            t2 = tmp.tile([128, NT], bf16, tag="rope_t2")
            nc.vector.tensor_mul(out=t2[:], in0=sw_ps[:], in1=sin_sb[:])
            nc.vector.tensor_add(out=out_ap, in0=t1[:], in1=t2[:])

        # ---- K projection + rope -> bounce ----
        k_bounce = dram.tile([HP, 128, NT], bf16)
        for hp in range(HP):
            acc = ps_acc.tile([128, NT], f32, tag="mm")
            for kt in range(KT):
                nc.tensor.matmul(acc[:], wqkv_sb[:, kt, C + hp * 128: C + (hp + 1) * 128],
                                 h_bf[:, kt, :], start=(kt == 0), stop=(kt == KT - 1))
            kraw = tmp.tile([128, NT], bf16, tag="praw")
            nc.scalar.activation(out=kraw[:], in_=acc[:], func=AF.Identity,
                                 bias=bk_sb[:, hp, :], scale=1.0)
            k_out = tmp.tile([128, NT], bf16, tag="k_out")
            rope(kraw, k_out[:])
            nc.sync.dma_start(out=k_bounce[hp], in_=k_out[:])

        k_ag = dram.tile([GROUP, HP, 128, NT], bf16)
        nc.gpsimd.collective_compute(
            "AllGather", OP.bypass, replica_groups=groups,
            ins=[k_bounce.opt()], outs=[k_ag.opt()])

        # ---- V projection (token-major, 65-col interleave w/ ones) ----
        v_sb = sb.tile([128, TT, H * 65], bf16, tag="F")
        v_65 = v_sb[:].rearrange("p t (h x) -> p t h x", x=65)
        nc.vector.memset(v_65[:, :, :, 64:65], 1.0)
        v_bounce = dram.tile([TT, 128, H * 65], bf16)
        for tt in range(TT):
            for ns in range(2):
                acc = ps_acc.tile([128, 512], f32, tag="mm")
                for kt in range(KT):
                    nc.tensor.matmul(acc[:], h_bf[:, kt, tt * 128:(tt + 1) * 128],
                                     wqkv_sb[:, kt, 2 * C + ns * 512: 2 * C + (ns + 1) * 512],
                                     start=(kt == 0), stop=False)
                nc.tensor.matmul(acc[:], ones_r[:], bv_sb[:, ns * 512:(ns + 1) * 512],
                                 start=False, stop=True)
                nc.scalar.activation(
                    out=v_65[:, tt, ns * 8:(ns + 1) * 8, 0:64],
                    in_=acc[:].rearrange("p (h x) -> p h x", x=64),
                    func=AF.Identity, bias=0.0, scale=1.0)
            nc.sync.dma_start(out=v_bounce[tt], in_=v_sb[:, tt, :])

        v_ag = dram.tile([GROUP, TT, 128, H * 65], bf16)
        nc.gpsimd.collective_compute(
            "AllGather", OP.bypass, replica_groups=groups,
            ins=[v_bounce.opt()], outs=[v_ag.opt()])

        # ---- gather K into SBUF (slot C) ----
        k_full = sb.tile([128, HP, GROUP, NT], bf16, tag="C")
        for hp in range(HP):
            for r in range(GROUP):
                nc.sync.dma_start(out=k_full[:, hp, r, :], in_=k_ag[r, hp])

        # ---- Q projection + rope (scale 1/8 folded into evacuation) ----
        q_all = sb.tile([128, HP, NT], bf16, tag="F")
        for hp in range(HP):
            acc = ps_acc.tile([128, NT], f32, tag="mm")
            for kt in range(KT):
                nc.tensor.matmul(acc[:], wqkv_sb[:, kt, hp * 128:(hp + 1) * 128],
                                 h_bf[:, kt, :], start=(kt == 0), stop=(kt == KT - 1))
            qraw = tmp.tile([128, NT], bf16, tag="praw")
            nc.scalar.activation(out=qraw[:], in_=acc[:], func=AF.Identity,
                                 bias=bq_sb[:, hp, :], scale=0.125)
            rope(qraw, q_all[:, hp, :])

        # ---- gather V into SBUF (slot A, after wqkv is done) ----
        v_full = sb.tile([128, NKT, H * 65], bf16, tag="A")
        for r in range(GROUP):
            for tt in range(TT):
                nc.sync.dma_start(out=v_full[:, r * TT + tt, :], in_=v_ag[r, tt])

        # ---- attention ----
        attn = sb.tile([128, HP, NT], bf16, tag="E")
        for h in range(H):
            hp, off = h // 2, (h % 2) * 64
            o_ps = ps_o.tile([65, NT], f32, tag="o")
            for kt in range(NKT):
                r, s = kt // TT, kt % TT
                s_ps = ps_acc.tile([128, NT], f32, tag="mm")
                nc.tensor.matmul(s_ps[:],
                                 k_full[off:off + 64, hp, r, s * 128:(s + 1) * 128],
                                 q_all[off:off + 64, hp, :], start=True, stop=True)
                p_t = pbuf.tile([128, NT], bf16, tag="p")
                nc.scalar.activation(out=p_t[:], in_=s_ps[:], func=AF.Exp,
                                     bias=0.0, scale=1.0)
                nc.tensor.matmul(o_ps[:], v_full[:, kt, h * 65:(h + 1) * 65],
                                 p_t[:], start=(kt == 0), stop=(kt == NKT - 1))
            rec = tmp.tile([1, NT], f32, tag="rec")
            nc.vector.reciprocal(out=rec[:], in_=o_ps[64:65, :])
            recb = tmp.tile([1, NT], bf16, tag="recb")
            nc.scalar.copy(out=recb[:], in_=rec[:])
            rb_ps = ps_b.tile([64, NT], f32, tag="b0")
            nc.tensor.matmul(rb_ps[:], ones_r[:, 0:64], recb[:], start=True, stop=True)
            o_sb = tmp.tile([64, NT], bf16, tag="o_sb")
            nc.scalar.copy(out=o_sb[:], in_=o_ps[0:64, :])
            nc.vector.tensor_mul(out=attn[off:off + 64, hp, :], in0=o_sb[:],
                                 in1=rb_ps[:])

        # ---- out projection + residual (y into slot C) ----
        y_sb = sb.tile([128, KT, NT], f32, tag="C")
        for mt in range(KT):
            wot = wst.tile([128, KT, 128], bf16, tag="wot")
            for kt in range(KT):
                nc.sync.dma_start(out=wot[:, kt, :], in_=wo[mt, kt])
            acc = ps_acc.tile([128, NT], f32, tag="mm")
            for kt in range(KT):
                nc.tensor.matmul(acc[:], wot[:, kt, :],
                                 attn[:, kt, :], start=(kt == 0), stop=(kt == KT - 1))
            t0 = tmp.tile([128, NT], f32, tag="evac")
            nc.scalar.activation(out=t0[:], in_=acc[:], func=AF.Identity,
                                 bias=bo_sb[:, mt, :], scale=1.0)
            nc.vector.tensor_add(out=y_sb[:, mt, :], in0=t0[:], in1=x_sb[:, mt, :])

        h2_bf = layernorm(y_sb, g2_sb, h2_sb, "X")

        # ---- MLP ----
        h1g = sb.tile([128, FT, NT], bf16, tag="A")
        for mt in range(FT):
            w1t = wst.tile([128, KT, 128], bf16, tag="w1t")
            for kt in range(KT):
                nc.sync.dma_start(out=w1t[:, kt, :], in_=w1[mt, kt])
            acc = ps_acc.tile([128, NT], f32, tag="mm")
            for kt in range(KT):
                nc.tensor.matmul(acc[:], w1t[:, kt, :], h2_bf[:, kt, :],
                                 start=(kt == 0), stop=(kt == KT - 1))
            nc.scalar.activation(out=h1g[:, mt, :], in_=acc[:], func=AF.Gelu,
                                 bias=b1_sb[:, mt, :], scale=1.0)
        for mt in range(KT):
            w2t = wst.tile([128, FT, 128], bf16, tag="w2t")
            for kt in range(FT):
                nc.sync.dma_start(out=w2t[:, kt, :], in_=w2[mt, kt])
            acc = ps_acc.tile([128, NT], f32, tag="mm")
            for kt in range(FT):
                nc.tensor.matmul(acc[:], w2t[:, kt, :], h1g[:, kt, :],
                                 start=(kt == 0), stop=(kt == FT - 1))
            t0 = tmp.tile([128, NT], f32, tag="evac")
            nc.scalar.activation(out=t0[:], in_=acc[:], func=AF.Identity,
                                 bias=b2_sb[:, mt, :], scale=1.0)
            nc.vector.tensor_add(out=t0[:], in0=t0[:], in1=y_sb[:, mt, :])
            nc.sync.dma_start(out=outT[mt], in_=t0[:])

    nc.compile()
    return nc


def _prep_inputs(inputs):
    """Full inputs -> list of 8 per-core input dicts."""
    g = {k: np.asarray(v, dtype=np.float32) for k, v in inputs.items()}
    xf = np.ascontiguousarray(g["x"].reshape(B * N, C))
    cosT = g["rotary_cos"].T  # [64, N]
    sinT = g["rotary_sin"].T

    wqkv = np.concatenate([g["Wq"], g["Wk"], g["Wv"]], axis=0).T  # [C, 3C]
    wqkv = np.ascontiguousarray(wqkv.reshape(KT, 128, 3 * C)).astype(BF)
    wo = np.ascontiguousarray(
        g["Wo"].T.reshape(KT, 128, KT, 128).transpose(2, 0, 1, 3)).astype(BF)
    w1 = np.ascontiguousarray(
        g["W1"].T.reshape(KT, 128, FT, 128).transpose(2, 0, 1, 3)).astype(BF)
    w2 = np.ascontiguousarray(
        g["W2"].T.reshape(FT, 128, KT, 128).transpose(2, 0, 1, 3)).astype(BF)

    pswap = np.zeros((128, 128), np.float32)
    for i in range(128):
        j = i + 32 if (i % 64) < 32 else i - 32
        pswap[i, j] = 1.0
    pswap = pswap.astype(BF)

    bqk = np.stack([(g["bq"] * 0.125).reshape(KT, 128, 1),
                    g["bk"].reshape(KT, 128, 1)]).astype(np.float32)
    shared = dict(
        wqkv=wqkv, wo=wo, w1=w1, w2=w2, pswap=pswap, bqk=bqk,
        bvr=g["bv"].reshape(1, C).astype(BF),
        bo=g["bo"].reshape(KT, 128, 1),
        b1=g["b1"].reshape(FT, 128, 1),
        b2=g["b2"].reshape(KT, 128, 1),
        g1=g["ln1_g"].reshape(KT, 128, 1),
        h1=g["ln1_b"].reshape(KT, 128, 1),
        g2=g["ln2_g"].reshape(KT, 128, 1),
        h2=g["ln2_b"].reshape(KT, 128, 1),
    )

    in_maps = []
    for i in range(NCORES):
        s = i * NT
        pos0 = (i % GROUP) * NT
        c = cosT[0:32, pos0:pos0 + NT]
        sn = sinT[0:32, pos0:pos0 + NT]
        m = dict(shared)
        m["xT"] = np.ascontiguousarray(xf[s:s + NT].T.reshape(KT, 128, NT))
        m["cosr"] = np.ascontiguousarray(np.tile(c, (4, 1))).astype(BF)
        m["sinr"] = np.ascontiguousarray(
            np.concatenate([-sn, sn, -sn, sn], axis=0)).astype(BF)
        in_maps.append(m)
    return in_maps


class _Runner:
    """Builds the shard_map'd jitted executable once; reusable across calls.

    Mirrors concourse.bass2jax.run_bass_via_pjrt's multi-core branch, kept
    separate so repeat calls skip retracing and so timing harnesses can feed
    pre-staged device arrays.
    """

    def __init__(self, nc):
        import jax
        import numpy as _np
        from jax.sharding import Mesh, PartitionSpec
        from jax.experimental.shard_map import shard_map
        import concourse.mybir as mybir
        from concourse.bass2jax import (
            _bass_exec_p, install_neuronx_cc_hook, partition_id_tensor)

        install_neuronx_cc_hook()
        self.jax = jax
        self.nc = nc
        partition_name = (nc.partition_id_tensor.name
                          if nc.partition_id_tensor else None)
        in_names, out_names, out_avals = [], [], []
        for alloc in nc.m.functions[0].allocations:
            if not isinstance(alloc, mybir.MemoryLocationSet):
                continue
            name = alloc.memorylocations[0].name
            if alloc.kind == "ExternalInput":
                if name != partition_name:
                    in_names.append(name)
            elif alloc.kind == "ExternalOutput":
                out_names.append(name)
                out_avals.append(jax.core.ShapedArray(
                    tuple(alloc.tensor_shape), mybir.dt.np(alloc.dtype)))
        self.n_params = len(in_names)
        self.in_names = list(in_names)
        self.out_names = out_names
        self.out_avals = out_avals
        all_in_names = in_names + out_names
        if partition_name is not None:
            all_in_names.append(partition_name)

        def _body(*args):
            operands = list(args)
            if partition_name is not None:
                operands.append(partition_id_tensor())
            return tuple(_bass_exec_p.bind(
                *operands,
                out_avals=tuple(out_avals),
                in_names=tuple(all_in_names),
                out_names=tuple(out_names),
                lowering_input_output_aliases=(),
                sim_require_finite=True,
                sim_require_nnan=True,
                nc=nc,
            ))

        devices = jax.devices()[:NCORES]
        self.mesh = Mesh(_np.asarray(devices), ("core",))
        n_outs = len(out_names)
        self.fn = jax.jit(
            shard_map(_body, mesh=self.mesh,
                      in_specs=(PartitionSpec("core"),) * (self.n_params + n_outs),
                      out_specs=(PartitionSpec("core"),) * n_outs,
                      check_rep=False),
            donate_argnums=tuple(range(self.n_params, self.n_params + n_outs)),
            keep_unused=True)

    def stage_inputs(self, in_maps):
        """Concatenate per-core inputs and move them to the devices."""
        import jax
        from jax.sharding import NamedSharding, PartitionSpec
        sh = NamedSharding(self.mesh, PartitionSpec("core"))
        concat = [np.concatenate([np.asarray(m[n]) for m in in_maps], axis=0)
                  for n in self.in_names]
        return [jax.device_put(a, sh) for a in concat]

    def fresh_outbufs(self):
        import jax
        from jax.sharding import NamedSharding, PartitionSpec
        sh = NamedSharding(self.mesh, PartitionSpec("core"))
        return [jax.device_put(
            np.zeros((NCORES * a.shape[0], *a.shape[1:]), a.dtype), sh)
            for a in self.out_avals]

    def run_staged(self, staged, outbufs):
        return self.fn(*staged, *outbufs)

    def run(self, in_maps):
        outs = self.fn(*self.stage_inputs(in_maps), *self.fresh_outbufs())
        return [
            {n: np.asarray(outs[i]).reshape(NCORES, *self.out_avals[i].shape)[c]
             for i, n in enumerate(self.out_names)}
            for c in range(NCORES)
        ]


def get_runner():
    if "runner" not in _CACHE:
        if "nc" not in _CACHE:
            _CACHE["nc"] = _build_program()
        _CACHE["runner"] = _Runner(_CACHE["nc"])
    return _CACHE["runner"]


def kernel(**inputs) -> np.ndarray:
    global LAST_RESULT
    runner = get_runner()
    results = runner.run(_prep_inputs(inputs))
    LAST_RESULT = results
    chunks = [results[i]["outT"].reshape(C, NT).T for i in range(NCORES)]
    return np.ascontiguousarray(
        np.concatenate(chunks, axis=0).reshape(B, N, C)).astype(np.float32)
